# revision 37
# baseline (speedup 1.0000x reference)
"""MRU encoding kernel for Trainium2 (8 NeuronCores, batch-parallel).

Problem (B=32, T=2048, D=300):
    z = tanh(x @ Wz.T + bz); o = tanh(x @ Wo.T + bo)
    c_t = g_t*c_{t-1} + (1-g_t)*z_t   (c_{-1}=0, scan over T)
    out = o * c

Per-core (4 batch rows) layout is [channel, time]:
  - host pre-transposes x,g to [b, D, T]; x gets a ones-row (301) so the
    bias rides in the matmul contraction; weights are fed as [D+1, 320] =
    [W.T; b] zero-padded so psum = W @ x + b directly in [e, t] layout.
  - o is produced NEGATED via tanh(scale=-1): with bneg=(g-1)*z = -(1-g)z
    the hardware scan state=g*state+bneg yields -c, and (-o)*(-c) = o*c.
  - the whole T=2048 recurrence per channel is ONE tensor_tensor_scan
    DVE instruction per 128-channel tile (state kept fp32 by HW).
  - the ragged e-rows 256..299 (44 of them, padded to 64 with zero weight
    columns) of TWO batch rows share one 128-partition tile: b_even at
    partitions 0..63, b_odd at 64..127 -> one tanh + one DVE chain per
    weight per pair instead of two.
  - the ragged k-chunk (45 rows) of Wo is duplicated at partition base 64
    so its matmuls row-tile concurrently with Wz's at base 0.
  - input loads ride the SP HWDGE ring; weights+stores ride the ACT ring
    (HWDGE is FIFO per issuing engine; stores must not block prefetch).
"""

import numpy as np

import concourse.bass as bass
import concourse.mybir as mybir
import concourse.tile as tile
from concourse import bacc
from concourse.bass_utils import run_bass_kernel_spmd

B, T, D = 32, 2048, 300
NCORES = 8
BC = B // NCORES  # 4 batch rows per core
DP = D + 1  # ones-row at index 300 carries the bias
DPAD = 320  # weight e-columns padded so the ragged chunk is m=64
TS = 512  # moving-operand max free dim
NT = T // TS
F32 = mybir.dt.float32
F32R = mybir.dt.float32r
F16 = mybir.dt.float16

KC = [(0, 128), (128, 128), (256, 45)]  # k-chunks (incl. ones row)

CFG = {"mm16": True, "plane16": True, "c16": True, "out16": True}

_CACHE: dict = {}


def _build_program(reps=1, bufs=None, cfg=None):
    c = dict(CFG)
    if cfg:
        c.update(cfg)
    mm_dt = F16 if c["mm16"] else F32R
    pl_dt = F16 if c["plane16"] else F32
    c_dt = F16 if c["c16"] else F32
    out_dt = F16 if c["out16"] else F32

    bf = {"xp": 2, "gp": 2, "zp": 1, "ep": 3, "ps": 2}
    if bufs:
        bf.update(bufs)

    nc = bacc.Bacc("TRN2", target_bir_lowering=False, debug=False, num_devices=NCORES)

    d_x = nc.dram_tensor("xt", [BC, DP, T], mm_dt, kind="ExternalInput").ap()
    d_g = nc.dram_tensor("gt", [BC, D, T], pl_dt, kind="ExternalInput").ap()
    d_wz = nc.dram_tensor("wz", [DP, DPAD], mm_dt, kind="ExternalInput").ap()
    d_wo = nc.dram_tensor("wo", [DP, DPAD], mm_dt, kind="ExternalInput").ap()
    # replicas share ONE output tensor: keeps the PJRT buffer count (and its
    # per-call overhead) constant across reps so marginal timing is clean
    d_out0 = nc.dram_tensor("outt", [BC, D, T], out_dt, kind="ExternalOutput").ap()
    d_outs = [d_out0] * reps

    with tile.TileContext(nc) as tc:
        with (
            tc.tile_pool(name="wp", bufs=1) as wp,
            tc.tile_pool(name="xp", bufs=bf["xp"]) as xp,
            tc.tile_pool(name="gp", bufs=bf["gp"]) as gp,
            tc.tile_pool(name="zp", bufs=bf["zp"]) as zp,
            tc.tile_pool(name="ep", bufs=bf["ep"]) as ep,
            tc.tile_pool(name="ps", bufs=bf["ps"], space="PSUM") as ps,
        ):
            # weights ride the scalar ring so they don't delay the first x load
            wts = {}
            for nm, dram in (("wz", d_wz), ("wo", d_wo)):
                t = wp.tile([128, 3, DPAD], mm_dt, tag=nm, name=f"w_{nm}")
                nc.scalar.dma_start(
                    t[:, 0:2, :], dram[0:256, :].rearrange("(c p) m -> p c m", c=2)
                )
                nc.scalar.dma_start(t[:45, 2, :], dram[256:DP, :])
                wts[nm] = t
            # wo ragged k-chunk duplicated at base 64 for row-tiling
            nc.scalar.dma_start(wts["wo"][64:109, 2, :], d_wo[256:DP, :])

            def elemwise(gs, z_ap, oneg_ap, mj, stores, tsplit=1):
                """bneg=(g-1)z -> scan(-c) -> out = (-o)*(-c); stores is a
                list of (res_slice, dram_slice). tsplit>1 pipelines the chain
                in T-chunks (scan chained via `initial`) so the final store
                overlaps the rest -- used for the kernel-tail chain."""
                bneg = ep.tile([128, T], pl_dt, tag="bneg", name="bneg_t")
                gm1 = None
                if c["plane16"]:
                    # TS(4x) + TT(2x) beats one STT(1x) at fp16
                    gm1 = ep.tile([128, T], pl_dt, tag="gm1", name="gm1_t")
                cneg = ep.tile([128, T], c_dt, tag="c", name="cneg_t")
                res = ep.tile([128, T], out_dt, tag="res", name="res_t")
                tw = T // tsplit
                for h in range(tsplit):
                    hs = slice(h * tw, (h + 1) * tw)
                    if gm1 is not None:
                        nc.vector.tensor_scalar_add(gm1[:mj, hs], gs[:, hs], -1.0)
                        nc.vector.tensor_mul(
                            bneg[:mj, hs], gm1[:mj, hs], z_ap[:, hs]
                        )
                    else:
                        nc.vector.scalar_tensor_tensor(
                            bneg[:mj, hs], gs[:, hs], 1.0, z_ap[:, hs],
                            op0=mybir.AluOpType.subtract,
                            op1=mybir.AluOpType.mult,
                        )
                    init = 0.0 if h == 0 else cneg[:mj, h * tw - 1 : h * tw]
                    nc.vector.tensor_tensor_scan(
                        cneg[:mj, hs], gs[:, hs], bneg[:mj, hs], init,
                        op0=mybir.AluOpType.mult, op1=mybir.AluOpType.add,
                    )
                    nc.vector.tensor_mul(
                        res[:mj, hs], oneg_ap[:, hs], cneg[:mj, hs]
                    )
                    for rs, ds in stores:
                        # stores ride the ACT HWDGE ring (never block prefetch)
                        nc.scalar.dma_start(
                            ds[:, hs], res[rs[0] : rs[1], hs]
                        )

            def proj_mms(pz, po, xt, msl, mj, zbase):
                """Matmul groups for one (m-slice, batch-row): z k-order
                0,1,2; o k-order 2,0,1 so both ragged k=45 passes sit
                adjacent and row-tile concurrently (wz rows 0..44, wo rows
                64..108). Outputs land at psum partitions zbase..zbase+mj."""
                for k in (0, 1):
                    kn = KC[k][1]
                    for tb in range(NT):
                        nc.tensor.matmul(
                            pz[zbase : zbase + mj, bass.ts(tb, TS)],
                            lhsT=wts["wz"][:kn, k, msl],
                            rhs=xt[:kn, k, bass.ts(tb, TS)],
                            start=(k == 0), stop=False,
                        )
                for tb in range(NT):
                    nc.tensor.matmul(
                        pz[zbase : zbase + mj, bass.ts(tb, TS)],
                        lhsT=wts["wz"][:45, 2, msl],
                        rhs=xt[:45, 2, bass.ts(tb, TS)],
                        start=False, stop=True,
                    )
                    nc.tensor.matmul(
                        po[zbase : zbase + mj, bass.ts(tb, TS)],
                        lhsT=wts["wo"][64:109, 2, msl],
                        rhs=xt[64:109, 2, bass.ts(tb, TS)],
                        start=True, stop=False,
                    )
                for k in (0, 1):
                    kn = KC[k][1]
                    for tb in range(NT):
                        nc.tensor.matmul(
                            po[zbase : zbase + mj, bass.ts(tb, TS)],
                            lhsT=wts["wo"][:kn, k, msl],
                            rhs=xt[:kn, k, bass.ts(tb, TS)],
                            start=False, stop=(k == 1),
                        )

            for d_out in d_outs:
              for pair in range(BC // 2):
                b0, b1 = 2 * pair, 2 * pair + 1
                xts = {}
                gts = {}
                for b in (b0, b1):
                    xt = xp.tile([128, 3, T], mm_dt, tag="x", name="xt_t")
                    nc.sync.dma_start(xt[:, 0, :], d_x[b, 0:128, :])
                    nc.sync.dma_start(xt[:, 1, :], d_x[b, 128:256, :])
                    nc.sync.dma_start(xt[:45, 2, :], d_x[b, 256:DP, :])
                    # duplicate ragged x chunk at base 64 for wo row-tiles
                    nc.sync.dma_start(xt[64:109, 2, :], d_x[b, 256:DP, :])
                    xts[b] = xt
                    gt = gp.tile([128, 2, T], pl_dt, tag="g", name="gt_t")
                    nc.sync.dma_start(
                        gt[:, :, :],
                        d_g[b, 0:256, :].rearrange("(c p) t -> p c t", c=2),
                    )
                    gts[b] = gt

                def do_j(b, j, tsplit=1):
                    m0 = 128 * j
                    pz = ps.tile([128, T], F32, tag="p", name="psum_z")
                    po = ps.tile([128, T], F32, tag="p", name="psum_o")
                    proj_mms(pz, po, xts[b], slice(m0, m0 + 128), 128, 0)
                    z_j = zp.tile([128, T], pl_dt, tag="z", name="t_z")
                    oneg_j = zp.tile([128, T], pl_dt, tag="o", name="t_o")
                    tw2 = T // tsplit
                    for h in range(tsplit):
                        hs = slice(h * tw2, (h + 1) * tw2)
                        nc.scalar.activation(
                            z_j[:, hs], pz[:, hs],
                            mybir.ActivationFunctionType.Tanh, scale=1.0,
                        )
                        nc.scalar.activation(
                            oneg_j[:, hs], po[:, hs],
                            mybir.ActivationFunctionType.Tanh, scale=-1.0,
                        )
                    elemwise(
                        gts[b][:, j, :], z_j[:, :], oneg_j[:, :], 128,
                        [((0, 128), d_out[b, m0 : m0 + 128, :])],
                        tsplit=tsplit,
                    )

                do_j(b0, 0)
                do_j(b0, 1)

                # ragged e-rows (padded to m=64) of BOTH batch rows share one
                # tile: b0 at partitions 0..63, b1 at 64..127
                g2 = gp.tile([128, T], pl_dt, tag="g2", name="g2_t")
                # pad lanes must stay finite for the scan; 32-aligned memsets
                # run first, the real loads overwrite their live lanes
                nc.gpsimd.memset(g2[32:64, :], 0.5)
                nc.gpsimd.memset(g2[96:128, :], 0.5)
                nc.sync.dma_start(g2[0:44, :], d_g[b0, 256:D, :])
                nc.sync.dma_start(g2[64:108, :], d_g[b1, 256:D, :])

                pz2 = ps.tile([128, T], F32, tag="p", name="psum_z2")
                po2 = ps.tile([128, T], F32, tag="p", name="psum_o2")
                proj_mms(pz2, po2, xts[b0], slice(256, DPAD), 64, 0)
                proj_mms(pz2, po2, xts[b1], slice(256, DPAD), 64, 64)
                z2 = zp.tile([128, T], pl_dt, tag="z", name="t_z2")
                nc.scalar.activation(
                    z2[:, :], pz2[:, :],
                    mybir.ActivationFunctionType.Tanh, scale=1.0,
                )
                oneg2 = zp.tile([128, T], pl_dt, tag="o", name="t_o2")
                nc.scalar.activation(
                    oneg2[:, :], po2[:, :],
                    mybir.ActivationFunctionType.Tanh, scale=-1.0,
                )
                elemwise(
                    g2[:, :], z2[:, :], oneg2[:, :], 128,
                    [((0, 44), d_out[b0, 256:D, :]),
                     ((64, 108), d_out[b1, 256:D, :])],
                )

                do_j(b1, 0)
                do_j(b1, 1)

    nc.compile()
    return nc


def kernel(gate_encoding, inputs_encoding, Wz, bz, Wo, bo):
    gate_encoding = np.asarray(gate_encoding, dtype=np.float32)
    inputs_encoding = np.asarray(inputs_encoding, dtype=np.float32)
    Wz = np.asarray(Wz, dtype=np.float32)
    bz = np.asarray(bz, dtype=np.float32)
    Wo = np.asarray(Wo, dtype=np.float32)
    bo = np.asarray(bo, dtype=np.float32)

    mm_np = np.float16 if CFG["mm16"] else np.float32
    pl_np = np.float16 if CFG["plane16"] else np.float32

    def aug(Wmat, bvec):
        w = np.zeros((DP, DPAD), dtype=np.float32)
        w[:D, :D] = Wmat.T
        w[D, :D] = bvec
        return w.astype(mm_np)

    wz_aug = aug(Wz, bz)
    wo_aug = aug(Wo, bo)

    if "nc" not in _CACHE:
        _CACHE["nc"] = _build_program()
    nc = _CACHE["nc"]

    in_maps = []
    for cc in range(NCORES):
        xs = inputs_encoding[cc * BC : (cc + 1) * BC]  # [BC, T, D]
        gs = gate_encoding[cc * BC : (cc + 1) * BC]
        xt = np.empty((BC, DP, T), dtype=mm_np)
        xt[:, :D, :] = xs.transpose(0, 2, 1)
        xt[:, D, :] = 1.0
        gt = gs.transpose(0, 2, 1).astype(pl_np)
        in_maps.append({"xt": xt, "gt": gt, "wz": wz_aug, "wo": wo_aug})

    res = run_bass_kernel_spmd(nc, in_maps, core_ids=list(range(NCORES)))

    out = np.empty((B, T, D), dtype=np.float32)
    for cc in range(NCORES):
        out[cc * BC : (cc + 1) * BC] = (
            res.results[cc]["outt"].transpose(0, 2, 1).astype(np.float32)
        )
    return out
